# revision 56
# baseline (speedup 1.0000x reference)
"""Trainium2 Bass kernel for nn_HardMemory (retrieval_knn).

For each spatial token (B*H*W tokens, C=128 channels), find the memory row
(of M=512) with max cosine similarity and replace the token's channel vector
with that raw memory row.

Algebraic simplification: argmax_m cos(x, mem_m) = argmax_m (x . mem_n_m)
where mem_n is the l2-normalized memory -- normalizing x is a positive
per-token scale and cannot change the argmax, so it is skipped.

Precision: scores are computed with a 3-term fp16 split
(s = xh.mh + xh.ml + xl.mh, fp32 PSUM accumulation); measured zero argmax
flips vs fp64 on the fixed seed. The normalized memory is pre-scaled by
BETA so PSUM holds beta*s directly; the one-hot is then
exp(beta*s - max(beta*s)) computed by the Activation engine in a single
pass (bias = -max as a per-partition AP; the subtraction is exact near
the top by Sterbenz cancellation, and runner-ups with beta*gap > ~17
flush to 0 in fp16). This replaces the old DVE is_ge pass and the
separate fp16 score copy.

The gather reconstructs fp16-rounded memory rows via one-hot matmuls
(4 fp16 matmuls per 512 tokens, M-chunked); output is stored fp16 and
upcast to fp32 on the host (quantization rel err ~1e-4, far under the
2e-2 gate).

Sharding: data-parallel over batch, 4 batches per core, memory replicated.

Per-core pipeline, per 2-tile (256-token) group, software-pipelined over 4
stages (head g | exp g-1 | transpose g-2 | gather g-4) so every cross-engine
dependency is at least one full group-iteration old when its consumer's
engine reaches it (a blocked wait serializes a whole sequencer under the
single-wait Drain legalization, so waits must be pre-satisfied):
  1. PE:  6x fp16 matmul -> beta*scores in one [128, 2, 512] PSUM tile
  2. DVE: one batched reduce_max (negate=True) -> -mx[tok, 2]
  3. ACT: onehot = exp(scores - mx) per tile (fp16, SBUF)
  4. PE:  4x 128x128 fp16 transpose per tile -> ohT[m, tok] (PSUM)
  5. DVE: copy ohT PSUM -> SBUF gather operand (2x fp16 mode)
  6. PE:  out[c, tok] = sum_k gm_k.T @ ohT_k (4 fp16 matmuls per 256 tok)
  7. ACT: copy gather PSUM -> SBUF fp16; DMA out; host upcasts to fp32

Input DMAs are sliced (independent tiles per slice) and spread one slice
per group to avoid serializing the 625ns-per-DMA HWDGE descriptor setups;
batch 0 loads a small 256-token head slice so PE starts ~5us earlier.
"""

import numpy as np

import concourse.bass as bass
import concourse.mybir as mybir
from concourse.tile import TileContext
from concourse.bass_utils import run_bass_kernel_spmd

F32 = mybir.dt.float32
F16 = mybir.dt.float16
AF = mybir.ActivationFunctionType

B, C, H, W = 32, 128, 64, 64
N = H * W              # 4096 tokens per batch
M = 512                # memory rows
NCORES = 8
BPC = B // NCORES      # batches per core
TOK = BPC * N          # tokens per core
TILE = 128             # tokens per tile
GRP = 2                # tiles per PSUM score group
LOAD = 4096            # tokens per input DMA chunk (one full batch image)
STORE = 256            # tokens per output DMA chunk
KCH = M // TILE        # 4 gather chunks


def _build():
    nc = bass.Bass(trn_type="TRN2")

    xh_in = nc.dram_tensor("xh", [BPC, C, N], F16, kind="ExternalInput")
    xl_in = nc.dram_tensor("xl", [BPC, C, N], F16, kind="ExternalInput")
    # beta-scaled mem-normalized-transposed [C, 2, M] fp16: [:,0]=hi [:,1]=lo
    mhl_in = nc.dram_tensor("mhl", [C, 2, M], F16, kind="ExternalInput")
    # raw memory rows fp16 [TILE, KCH, C] + identity [TILE, TILE], packed
    gmi_in = nc.dram_tensor(
        "gmi", [TILE, KCH * C + TILE], F16, kind="ExternalInput"
    )
    out_d = nc.dram_tensor("out", [BPC, C, N], F16, kind="ExternalOutput")

    with TileContext(nc) as tc:
        with (
            tc.tile_pool(name="const", bufs=1) as cpool,
            tc.tile_pool(name="xin", bufs=4) as xpool,
            tc.tile_pool(name="oh", bufs=8) as ohpool,
            tc.tile_pool(name="oht", bufs=4) as ohtpool,
            tc.tile_pool(name="osb", bufs=4) as opool,
            tc.tile_pool(name="small", bufs=6) as spool,
            tc.tile_pool(name="ps_s", bufs=2, space="PSUM") as ps_s,
            tc.tile_pool(name="ps_t", bufs=2, space="PSUM") as ps_t,
            tc.tile_pool(name="ps_o", bufs=2, space="PSUM") as ps_o,
        ):
            n_groups = TOK // (TILE * GRP)
            grp_per_batch = N // (TILE * GRP)
            loaded = {}

            def load_slice(b, s0, s1):
                """One xh+xl slice of batch b into its own tiles (no false
                whole-tile deps; stores interleave between slice DMAs)."""
                nb = 8 if s1 - s0 == 1024 else 1
                xh_sb = xpool.tile(
                    [C, s1 - s0], F16, tag=f"xh{s1 - s0}", bufs=nb
                )
                nc.sync.dma_start(out=xh_sb, in_=xh_in[b, :, s0:s1])
                xl_sb = xpool.tile(
                    [C, s1 - s0], F16, tag=f"xl{s1 - s0}", bufs=nb
                )
                nc.sync.dma_start(out=xl_sb, in_=xl_in[b, :, s0:s1])
                loaded.setdefault(b, []).append((s0, s1, xh_sb, xl_sb))

            def xslice(b, o, size):
                for s0, s1, xh_sb, xl_sb in loaded[b]:
                    if s0 <= o and o + size <= s1:
                        return xh_sb[:, o - s0 : o - s0 + size], \
                               xl_sb[:, o - s0 : o - s0 + size]
                raise AssertionError((b, o, size))

            # Prefetch schedule: batches 1..3 load one 1024-token slice per
            # group, starting 12 groups before the batch is needed, so HWDGE
            # descriptor setup (625ns/DMA, serialized) never bursts.
            load_plan = {}
            for b in range(1, BPC):
                for si in range(4):
                    load_plan.setdefault(
                        b * grp_per_batch - 12 + 2 * si, []
                    ).append((b, si * 1024, (si + 1) * 1024))

            mhl = cpool.tile([C, 2, M], F16)
            nc.sync.dma_start(out=mhl, in_=mhl_in[:])
            mh = mhl[:, 0, :]
            ml = mhl[:, 1, :]
            # batch 0: small leading slices so PE starts early
            for s0, s1 in ((0, 256), (256, 1024), (1024, 2048), (2048, 3072),
                           (3072, N)):
                load_slice(0, s0, s1)
            gmi = cpool.tile([TILE, KCH * C + TILE], F16)
            nc.sync.dma_start(out=gmi, in_=gmi_in[:])
            gm = gmi[:, : KCH * C].rearrange("p (k c) -> p k c", k=KCH)
            ident = gmi[:, KCH * C :]

            def head(g):
                """Score matmuls for group g + batched negated max."""
                gtok0 = g * TILE * GRP
                b = gtok0 // N
                for lb, s0, s1 in load_plan.get(g, ()):
                    load_slice(lb, s0, s1)
                ps4 = ps_s.tile([TILE, GRP, M], F32)
                for j in range(GRP):
                    tok0 = gtok0 + j * TILE
                    o = tok0 % LOAD
                    xht, xlt = xslice(b, o, TILE)
                    ps = ps4[:, j, :]
                    nc.tensor.matmul(out=ps, lhsT=xht, rhs=mh, start=True, stop=False)
                    nc.tensor.matmul(out=ps, lhsT=xht, rhs=ml, start=False, stop=False)
                    nc.tensor.matmul(out=ps, lhsT=xlt, rhs=mh, start=False, stop=True)
                nbmx = spool.tile([TILE, GRP], F32, tag="nbmx")
                nc.vector.reduce_max(
                    out=nbmx, in_=ps4, axis=mybir.AxisListType.X, negate=True
                )
                return ps4, nbmx

            def exp_stage(g, ps4, nbmx):
                """Exp one-hot for group g (1 group late)."""
                ohs = []
                for j in range(GRP):
                    oh = ohpool.tile([TILE, M], F16)
                    nc.scalar.activation(
                        out=oh, in_=ps4[:, j, :], func=AF.Exp,
                        bias=nbmx[:, j : j + 1], scale=1.0,
                    )
                    ohs.append(oh)
                return ohs

            def tr_stage(g, ohs):
                """Transpose + copy into gather operand for group g (2 groups
                late, so the exp output is guaranteed ready and PE never
                stalls on ACT)."""
                gtok0 = g * TILE * GRP
                oht = ohtpool.tile([TILE, KCH, STORE], F16)
                for j in range(GRP):
                    tok0 = gtok0 + j * TILE
                    oh = ohs[j]
                    oht_ps = ps_t.tile([TILE, M], F16)
                    for k in range(KCH):
                        nc.tensor.transpose(
                            out=oht_ps[:, k * TILE : (k + 1) * TILE],
                            in_=oh[:, k * TILE : (k + 1) * TILE],
                            identity=ident,
                        )
                    off = tok0 % STORE
                    src = oht_ps.rearrange("p (k t) -> p k t", k=KCH)
                    dst = oht[:, :, off : off + TILE]
                    nc.vector.tensor_copy(dst, src)
                return oht

            def gtail(g, oht):
                """Gather + out-copy + store for group g (3 groups late so
                the DVE copies it reads are long done)."""
                gtok0 = g * TILE * GRP
                tok0 = gtok0 + (GRP - 1) * TILE
                b, n0 = divmod(tok0, N)
                po = ps_o.tile([C, STORE], F32)
                for k in range(KCH):
                    nc.tensor.matmul(
                        out=po,
                        lhsT=gm[:, k, :],
                        rhs=oht[:, k, :],
                        start=(k == 0),
                        stop=(k == KCH - 1),
                    )
                ob = opool.tile([C, STORE], F16)
                nc.scalar.activation(out=ob, in_=po, func=AF.Copy)
                nc.sync.dma_start(
                    out=out_d[b, :, n0 + TILE - STORE : n0 + TILE],
                    in_=ob,
                )

            assert STORE == GRP * TILE
            # 4-stage software pipeline: head(g) | exp(g-1) | transpose(g-2)
            # | gather(g-3). Each stage's inputs are >=1 full group-iteration
            # old when its engine reaches them, so no engine ever stalls.
            p_head = p_exp = None
            trs = {}
            for g in range(n_groups + 4):
                nxt_head = head(g) if g < n_groups else None
                nxt_exp = exp_stage(g - 1, *p_head) if p_head is not None else None
                if p_exp is not None:
                    trs[g - 2] = tr_stage(g - 2, p_exp)
                if g - 4 in trs:
                    gtail(g - 4, trs.pop(g - 4))
                p_head, p_exp = nxt_head, nxt_exp

    _legalize_waits(nc)
    nc.finalize()
    return nc


def _legalize_waits(nc):
    """This container's walrus accepts only ONE sync wait per engine
    instruction (setupSyncWait: 'Too many sync wait commands'). Tile emits
    multi-wait instructions (and an 11-wait tail drain). Split: keep one
    wait on the instruction, hoist the rest onto single-wait Drain ops
    inserted just before it on the same engine (engine order preserved =>
    semantics preserved). DMA copies are left alone (ring descriptors
    accept multiple waits)."""
    n_split = 0
    for f in nc.m.functions:
        for b in f.blocks:
            out = []
            for inst in b.instructions:
                si = inst.sync_info
                if si is not None and len(si.on_wait) > 1:
                    waits = list(si.on_wait)
                    for j, w in enumerate(waits[:-1]):
                        out.append(
                            mybir.InstDrain(
                                name=f"{inst.name}-w{j}",
                                engine=inst.engine,
                                ins=[],
                                outs=[],
                                sync_info=mybir.SyncInfo(
                                    on_wait=[w], on_update=[]
                                ),
                            )
                        )
                    inst.sync_info = mybir.SyncInfo(
                        on_wait=[waits[-1]], on_update=list(si.on_update)
                    )
                    n_split += 1
                out.append(inst)
            b.instructions = out
    return n_split


_NC = None


def _get_nc():
    global _NC
    if _NC is None:
        _NC = _build()
    return _NC


def _host_prep(x, memory):
    memn = memory / np.maximum(
        np.sqrt((memory * memory).sum(axis=1, keepdims=True)), 1e-12
    )
    # Scale the normalized memory by BETA so PSUM holds beta*s directly.
    # Cap so the fp16 constants can't overflow (|mh| <= ~55000 < 65504).
    beta = min(1e5, 55000.0 / max(float(np.abs(memn).max()), 1e-6))
    mnt = np.ascontiguousarray(memn.T).astype(np.float32) * beta   # [C, M]
    mh = mnt.astype(np.float16)
    ml = (mnt - mh.astype(np.float32)).astype(np.float16)
    mhl = np.stack([mh, ml], axis=1)                               # [C, 2, M]

    gh = memory.astype(np.float16)
    gmi = np.empty((TILE, KCH * C + TILE), dtype=np.float16)
    for k in range(KCH):
        gmi[:, k * C : (k + 1) * C] = gh[k * TILE : (k + 1) * TILE, :]
    gmi[:, KCH * C :] = np.eye(TILE, dtype=np.float16)

    xh = x.astype(np.float16)
    xl = (x - xh.astype(np.float32)).astype(np.float16)
    return xh, xl, mhl, gmi


def kernel(x, memory):
    x = np.asarray(x, dtype=np.float32)
    memory = np.asarray(memory, dtype=np.float32)
    nc = _get_nc()
    xh, xl, mhl, gmi = _host_prep(x, memory)

    in_maps = []
    for c in range(NCORES):
        in_maps.append({
            "xh": np.ascontiguousarray(xh[c * BPC : (c + 1) * BPC].reshape(BPC, C, N)),
            "xl": np.ascontiguousarray(xl[c * BPC : (c + 1) * BPC].reshape(BPC, C, N)),
            "mhl": mhl, "gmi": gmi,
        })

    res = run_bass_kernel_spmd(nc, in_maps, core_ids=list(range(NCORES)))
    outs = [
        r["out"].astype(np.float32).reshape(BPC, C, H, W) for r in res.results
    ]
    return np.concatenate(outs, axis=0)
